# revision 27
# baseline (speedup 1.0000x reference)
"""GENConv-style message passing + MLP head on 8 Trainium2 NeuronCores (Bass/Tile).

Sharding: destination nodes across 8 cores (each core owns its contiguous node
rows and its contiguous edge block). Weights replicated. The per-edge source
features arrive as a host-prepared shard layout: each core's input includes
x.T columns replicated per edge (xg[:, e] = x[src[e], :]), so the device
computes x_j = w_src @ xg[:, e] with a plain streaming matmul — no indirect
DMA (measured ~8 ns/row descriptor generation on the Q7 makes device-side row
gathers ~1.6 ms/core, far slower than streaming the expanded columns).

Edge stage (layout: channels on partitions, edges on the free dim, two
512-edge tiles stacked on partition halves so DVE/ACT run 128 lanes wide):
  v = w_src@xg + blockdiag(w_edge)@ea2 accumulated in PSUM (three matmuls per
  1024 edges), m = relu(v) and a = exp(m) on ACT, p = m*a on DVE, then
  per-node sums over K=32 edges are free-dim segmented DVE reductions.
  Softmax uses the no-max identity out = sum(m e^m)/sum(e^m) + eps + dst
  (exact: exp cannot overflow here, and the eps/gmax factors cancel).
  Invalid and padded edges have xg and ea zeroed, so they contribute m=0,
  a=1, p=0; the spurious a=1 counts are subtracted per node via a
  host-computed count.

MLP head: h = out @ w1.T with train-mode BatchNorm using global batch stats
(partial sums AllReduce'd across the 8 cores), relu, then @ w2.T.
"""

import numpy as np
import ml_dtypes

import concourse.bacc as bacc
import concourse.bass as bass
import concourse.mybir as mybir
import concourse.tile as tile
from concourse.bass_utils import run_bass_kernel_spmd

BF16 = ml_dtypes.bfloat16
dt = mybir.dt
P = 128

N, K, IN_C, H, ED = 50000, 32, 128, 64, 32
NCORES = 8
MSG_EPS = 1e-7
BN_EPS = 1e-5
TP = 1024  # edges per pair-tile (two 512-edge halves stacked on partitions)


def _geom(n, ncores):
    nb = n // ncores
    ngrp = -(-nb // 64)        # node groups of 64 (= one 2048-edge super-tile)
    np_pad = ngrp * 64
    return dict(n=n, ncores=ncores, nb=nb, np_pad=np_pad, ntile=ngrp,
                ep=np_pad * K)


def _mlp_chunks(nb):
    out = []
    c = 0
    while c < nb:
        out.append((c, min(512, nb - c)))
        c += 512
    return out


def build_program(g):
    n, ncores, nb, np_pad = g["n"], g["ncores"], g["nb"], g["np_pad"]
    ntile, ep = g["ntile"], g["ep"]

    nc = bacc.Bacc(None, target_bir_lowering=False, num_devices=ncores)

    xg_h = nc.dram_tensor("xg", [IN_C, ep], dt.bfloat16, kind="ExternalInput")
    ea2_h = nc.dram_tensor("ea2", [2 * ED, ep // 2], dt.bfloat16,
                           kind="ExternalInput")
    xTd_h = nc.dram_tensor("xTd", [IN_C, np_pad], dt.bfloat16,
                           kind="ExternalInput")
    cnt_h = nc.dram_tensor("cnt", [P, np_pad // 2], dt.float32,
                           kind="ExternalInput")
    wsrc_h = nc.dram_tensor("wsrcT", [IN_C, H], dt.bfloat16,
                            kind="ExternalInput")
    wdst_h = nc.dram_tensor("wdstT", [IN_C, H], dt.bfloat16,
                            kind="ExternalInput")
    wbd_h = nc.dram_tensor("wbd2", [2 * ED, P], dt.bfloat16,
                           kind="ExternalInput")
    w1_h = nc.dram_tensor("w1T", [H, 2 * H], dt.bfloat16, kind="ExternalInput")
    w2_h = nc.dram_tensor("w2T", [2 * H, H], dt.bfloat16, kind="ExternalInput")
    gam_h = nc.dram_tensor("gamma", [2 * H, 1], dt.float32,
                           kind="ExternalInput")
    bet_h = nc.dram_tensor("beta", [2 * H, 1], dt.float32, kind="ExternalInput")
    y_h = nc.dram_tensor("y", [H, np_pad], dt.float32, kind="ExternalOutput")

    chunks = _mlp_chunks(nb)
    nch = len(chunks)

    with tile.TileContext(nc) as tc:
        with tc.tile_pool(name="const", bufs=1) as cpool, \
             tc.tile_pool(name="xgin", bufs=3) as xgpool, \
             tc.tile_pool(name="eain", bufs=3) as eapool, \
             tc.tile_pool(name="work", bufs=8) as wpool, \
             tc.tile_pool(name="node", bufs=4) as npool, \
             tc.tile_pool(name="psV", bufs=3, space="PSUM") as psV, \
             tc.tile_pool(name="psB", bufs=2, space="PSUM") as psB, \
             tc.tile_pool(name="dram", bufs=1, space="DRAM") as dpool:

            # ---- resident constants ----
            wsrc_t = cpool.tile([IN_C, H], dt.bfloat16)
            wdst_t = cpool.tile([IN_C, H], dt.bfloat16)
            wbd_t = cpool.tile([2 * ED, P], dt.bfloat16)
            w1_t = cpool.tile([H, 2 * H], dt.bfloat16)
            w2_t = cpool.tile([2 * H, H], dt.bfloat16)
            gam_t = cpool.tile([2 * H, 1], dt.float32)
            bet_t = cpool.tile([2 * H, 1], dt.float32)
            eps_t = cpool.tile([P, 1], dt.float32)
            bne_t = cpool.tile([P, 1], dt.float32)
            nc.sync.dma_start(wsrc_t[:], wsrc_h[:])
            nc.sync.dma_start(wdst_t[:], wdst_h[:])
            nc.sync.dma_start(wbd_t[:], wbd_h[:])
            nc.sync.dma_start(w1_t[:], w1_h[:])
            nc.sync.dma_start(w2_t[:], w2_h[:])
            nc.sync.dma_start(gam_t[:], gam_h[:])
            nc.sync.dma_start(bet_t[:], bet_h[:])
            nc.vector.memset(eps_t[:], MSG_EPS)
            nc.vector.memset(bne_t[:], BN_EPS)

            cnt_sb = cpool.tile([P, np_pad // 2], dt.float32)  # stacked counts
            dstk_sb = cpool.tile([P, np_pad // 2], dt.float32)  # denom stacked
            nstk_sb = cpool.tile([P, np_pad // 2], dt.float32)  # num stacked
            ostk_sb = cpool.tile([P, np_pad // 2], dt.bfloat16)  # out stacked
            outT_sb = cpool.tile([H, np_pad], dt.bfloat16)
            h_sb = cpool.tile([P, nb], dt.bfloat16)
            nc.sync.dma_start(cnt_sb[:], cnt_h[:])

            bnc_in = dpool.tile([2 * H, 2], dt.float32)
            bnc_out = dpool.tile([2 * H, 2], dt.float32)

            # ---- dst features (+ eps) written straight into outT ----
            c = 0
            while c < np_pad:
                w = min(512, np_pad - c)
                xd = xgpool.tile([IN_C, 512], dt.bfloat16, tag="xd")
                nc.sync.dma_start(xd[:, :w], xTd_h[:, c:c + w])
                ps = psB.tile([P, 512], dt.float32, tag="big")
                nc.tensor.matmul(ps[0:H, :w], lhsT=wdst_t[:], rhs=xd[:, :w],
                                 start=True, stop=True)
                nc.scalar.activation(outT_sb[:, c:c + w], ps[0:H, :w],
                                     mybir.ActivationFunctionType.Identity,
                                     bias=eps_t[0:H, 0:1])
                c += w

            # ---- edge stage: one super-tile = 2048 edges = 64 nodes ----
            T2 = 2 * TP
            for t in range(ntile):
                e0 = t * T2
                xgt = xgpool.tile([IN_C, T2], dt.bfloat16, tag="xg")
                nc.sync.dma_start(xgt[:], xg_h[:, e0:e0 + T2])
                eat = eapool.tile([2 * ED, TP], dt.bfloat16, tag="ea")
                nc.sync.dma_start(eat[:], ea2_h[:, e0 // 2:(e0 + T2) // 2])

                vps = psV.tile([P, 1024], dt.float32, tag="v")
                for hh in range(2):
                    o5, o10 = hh * 512, hh * 1024
                    nc.tensor.matmul(vps[:, o5:o5 + 512],
                                     lhsT=wbd_t[:], rhs=eat[:, o5:o5 + 512],
                                     start=True, stop=False,
                                     skip_group_check=True)
                    nc.tensor.matmul(vps[0:H, o5:o5 + 512], lhsT=wsrc_t[:],
                                     rhs=xgt[:, o10:o10 + 512], start=False,
                                     stop=False, skip_group_check=True)
                    nc.tensor.matmul(vps[H:P, o5:o5 + 512], lhsT=wsrc_t[:],
                                     rhs=xgt[:, o10 + 512:o10 + 1024],
                                     start=False, stop=True,
                                     skip_group_check=True)

                m_t = wpool.tile([P, 1024], dt.bfloat16, tag="m")
                nc.scalar.activation(m_t[:], vps[:],
                                     mybir.ActivationFunctionType.Relu)
                a_t = wpool.tile([P, 1024], dt.bfloat16, tag="a")
                nc.scalar.activation(a_t[:], m_t[:],
                                     mybir.ActivationFunctionType.Exp)
                p_t = wpool.tile([P, 1024], dt.bfloat16, tag="p")
                nc.vector.tensor_tensor(out=p_t[:], in0=m_t[:], in1=a_t[:],
                                        op=mybir.AluOpType.mult)
                c0 = t * 32
                nc.vector.reduce_sum(dstk_sb[:, c0:c0 + 32],
                                     a_t[:].rearrange("p (s k) -> p s k", k=K),
                                     axis=mybir.AxisListType.X)
                nc.vector.reduce_sum(nstk_sb[:, c0:c0 + 32],
                                     p_t[:].rearrange("p (s k) -> p s k", k=K),
                                     axis=mybir.AxisListType.X)

            # ---- node stage in stacked layout ----
            c = 0
            while c < np_pad // 2:
                w = min(512, np_pad // 2 - c)
                dn = npool.tile([P, 512], dt.float32, tag="dn")
                nc.vector.tensor_tensor(out=dn[:, :w], in0=dstk_sb[:, c:c + w],
                                        in1=cnt_sb[:, c:c + w],
                                        op=mybir.AluOpType.subtract)
                rr = npool.tile([P, 512], dt.float32, tag="rr")
                nc.vector.reciprocal(rr[:, :w], dn[:, :w])
                nc.vector.tensor_tensor(out=ostk_sb[:, c:c + w],
                                        in0=nstk_sb[:, c:c + w],
                                        in1=rr[:, :w], op=mybir.AluOpType.mult)
                c += w

            # ---- unstack node sums, then add into outT (dst already there) ----
            ou_sb = cpool.tile([H, np_pad], dt.bfloat16)
            for src_row, off in ((0, 0), (H, 16)):
                nc.sync.dma_start(
                    ou_sb[:].rearrange("h (t s) -> h t s", s=32)[:, :, off:off + 16],
                    ostk_sb[src_row:src_row + H, :].rearrange(
                        "h (t s) -> h t s", s=16))
            c = 0
            while c < np_pad:
                w = min(512, np_pad - c)
                nc.vector.tensor_tensor(out=outT_sb[:, c:c + w],
                                        in0=outT_sb[:, c:c + w],
                                        in1=ou_sb[:, c:c + w],
                                        op=mybir.AluOpType.add)
                c += w

            # ---- MLP + BatchNorm (global stats via AllReduce) ----
            st_s = cpool.tile([P, nch], dt.float32)
            st_q = cpool.tile([P, nch], dt.float32)
            for ci, (c, w) in enumerate(chunks):
                hps = psB.tile([P, 512], dt.float32, tag="big")
                nc.tensor.matmul(hps[:, :w], lhsT=w1_t[:],
                                 rhs=outT_sb[:, c:c + w], start=True, stop=True)
                nc.scalar.copy(h_sb[:, c:c + w], hps[:, :w])
                nc.vector.reduce_sum(st_s[:, ci:ci + 1], hps[:, :w],
                                     axis=mybir.AxisListType.X)
                sqd = wpool.tile([P, 512], dt.bfloat16, tag="sqd")
                nc.scalar.activation(sqd[:, :w], hps[:, :w],
                                     mybir.ActivationFunctionType.Square,
                                     accum_out=st_q[:, ci:ci + 1])
            pk = cpool.tile([P, 2], dt.float32)
            nc.vector.reduce_sum(pk[:, 0:1], st_s[:, 0:nch],
                                 axis=mybir.AxisListType.X)
            nc.vector.reduce_sum(pk[:, 1:2], st_q[:, 0:nch],
                                 axis=mybir.AxisListType.X)
            nc.gpsimd.dma_start(bnc_in[:], pk[:])
            nc.gpsimd.collective_compute(
                "AllReduce", mybir.AluOpType.add,
                replica_groups=[list(range(ncores))],
                ins=[bnc_in.opt()], outs=[bnc_out.opt()],
            )
            sg = cpool.tile([P, 2], dt.float32)
            nc.gpsimd.dma_start(sg[:], bnc_out[:])

            mean = cpool.tile([P, 1], dt.float32)
            ex2 = cpool.tile([P, 1], dt.float32)
            var = cpool.tile([P, 1], dt.float32)
            sd = cpool.tile([P, 1], dt.float32)
            inv = cpool.tile([P, 1], dt.float32)
            scl = cpool.tile([P, 1], dt.float32)
            tmp = cpool.tile([P, 1], dt.float32)
            shf = cpool.tile([P, 1], dt.float32)
            nc.vector.tensor_scalar_mul(mean[:], sg[:, 0:1], 1.0 / n)
            nc.vector.tensor_scalar_mul(ex2[:], sg[:, 1:2], 1.0 / n)
            nc.vector.tensor_tensor(out=tmp[:], in0=mean[:], in1=mean[:],
                                    op=mybir.AluOpType.mult)
            nc.vector.tensor_tensor(out=var[:], in0=ex2[:], in1=tmp[:],
                                    op=mybir.AluOpType.subtract)
            nc.scalar.activation(sd[:], var[:],
                                 mybir.ActivationFunctionType.Sqrt,
                                 bias=bne_t[:, 0:1])
            nc.vector.reciprocal(inv[:], sd[:])
            nc.vector.tensor_tensor(out=scl[:], in0=gam_t[:], in1=inv[:],
                                    op=mybir.AluOpType.mult)
            nc.vector.tensor_tensor(out=tmp[:], in0=mean[:], in1=scl[:],
                                    op=mybir.AluOpType.mult)
            nc.vector.tensor_tensor(out=shf[:], in0=bet_t[:], in1=tmp[:],
                                    op=mybir.AluOpType.subtract)

            for ci, (c, w) in enumerate(chunks):
                hr = wpool.tile([P, 512], dt.bfloat16, tag="hr")
                nc.scalar.activation(hr[:, :w], h_sb[:, c:c + w],
                                     mybir.ActivationFunctionType.Relu,
                                     scale=scl[:, 0:1], bias=shf[:, 0:1])
                yps = psB.tile([P, 512], dt.float32, tag="big")
                nc.tensor.matmul(yps[0:H, :w], lhsT=w2_t[:], rhs=hr[:, :w],
                                 start=True, stop=True)
                yo = npool.tile([H, 512], dt.float32, tag="yo")
                nc.vector.tensor_copy(yo[:, :w], yps[0:H, :w])
                nc.sync.dma_start(y_h[:, c:c + w], yo[:, :w])

    nc.compile()
    return nc


def prep_inputs(g, x, edge_attr, w_src, w_dst, w_edge, w1, gamma, beta, w2,
                edge_index, nbr):
    """Host-side shard prep. Returns list of per-core input dicts."""
    n, ncores, nb, np_pad = g["n"], g["ncores"], g["nb"], g["np_pad"]
    ep = g["ep"]
    k = nbr.shape[1]

    x = np.asarray(x, np.float32)
    edge_attr = np.asarray(edge_attr, np.float32)
    nbr = np.asarray(nbr)
    src_n = np.asarray(edge_index[0])

    # The shard layout relies on the consistent-graph structure from
    # setup_inputs: edge e has destination e // K and nbr[n] lists node n's
    # contiguous edge ids (-1-padded). Fail loudly if that ever changes.
    e_ids = np.arange(nbr.size, dtype=nbr.dtype).reshape(nbr.shape)
    valid = nbr >= 0
    assert np.array_equal(np.where(valid, nbr, -1), np.where(valid, e_ids, -1)), \
        "nbr is not the contiguous edge-id layout this kernel assumes"
    dst_n = np.asarray(edge_index[1])
    assert np.array_equal(dst_n[:: nbr.shape[1]],
                          np.arange(nbr.shape[0], dtype=dst_n.dtype)), \
        "edge destinations are not grouped contiguously by node"
    deg = valid.sum(1).astype(np.float32)
    cnt = (k - deg).astype(np.float32)

    xT = np.ascontiguousarray(x.T).astype(BF16)
    wsrcT = np.ascontiguousarray(w_src.T).astype(BF16)
    wdstT = np.ascontiguousarray(w_dst.T).astype(BF16)
    w1T = np.ascontiguousarray(np.asarray(w1).T).astype(BF16)
    w2T = np.ascontiguousarray(np.asarray(w2).T).astype(BF16)
    weT = np.asarray(w_edge, np.float32).T          # [ED, H]
    wbd2 = np.zeros((2 * ED, P), np.float32)
    wbd2[0:ED, 0:H] = weT
    wbd2[ED:2 * ED, H:P] = weT
    wbd2 = wbd2.astype(BF16)
    gam = np.asarray(gamma, np.float32).reshape(2 * H, 1)
    bet = np.asarray(beta, np.float32).reshape(2 * H, 1)

    maps = []
    for i in range(ncores):
        n0 = i * nb
        val_b = valid[n0:n0 + nb].reshape(-1)
        src_b = np.asarray(src_n[n0 * k:(n0 + nb) * k], np.int64)

        # xg: per-edge source columns, zeroed for invalid/padded edges
        src_pad = np.zeros(ep, np.int64)
        src_pad[:nb * k] = np.where(val_b, src_b, 0)
        xg = xT[:, src_pad]
        xg[:, :nb * k][:, ~val_b] = 0
        xg[:, nb * k:] = 0

        ea_b = np.zeros((ep, ED), np.float32)
        ea_b[:nb * k] = np.where(val_b[:, None],
                                 edge_attr[n0 * k:(n0 + nb) * k], 0.0)
        # stacked pairs: col (u*512+c) rows 0:32 = ea[1024u+c], 32:64 = +512
        ea2 = np.ascontiguousarray(
            ea_b.reshape(-1, 2, 512, ED).transpose(1, 3, 0, 2).reshape(
                2 * ED, ep // 2)).astype(BF16)

        cnt_core = np.zeros(np_pad, np.float32)
        cnt_core[:nb] = cnt[n0:n0 + nb]
        cc = cnt_core.reshape(-1, 2, 16)  # [t, half, s]
        cntT = np.ascontiguousarray(np.concatenate([
            np.broadcast_to(cc[:, 0, :].reshape(1, -1), (H, np_pad // 2)),
            np.broadcast_to(cc[:, 1, :].reshape(1, -1), (H, np_pad // 2)),
        ], axis=0)).astype(np.float32)

        xTd = np.zeros((IN_C, np_pad), BF16)
        xTd[:, :nb] = xT[:, n0:n0 + nb]

        maps.append({
            "xg": np.ascontiguousarray(xg), "ea2": ea2, "xTd": xTd,
            "cnt": cntT, "wsrcT": wsrcT, "wdstT": wdstT, "wbd2": wbd2,
            "w1T": w1T, "w2T": w2T, "gamma": gam, "beta": bet,
        })
    return maps


def kernel(x, edge_attr, w_src, w_dst, w_edge, w1, gamma, beta, w2, edge_index,
           nbr):
    import os
    g = _geom(N, NCORES)
    maps = prep_inputs(g, x, edge_attr, w_src, w_dst, w_edge, w1, gamma, beta,
                       w2, edge_index, nbr)
    nc = build_program(g)
    trace = bool(os.environ.get("GEN_KERNEL_TRACE"))
    res = run_bass_kernel_spmd(nc, maps, core_ids=list(range(NCORES)),
                               trace=trace)
    if trace:
        kernel.last_result = res
    out = np.concatenate(
        [np.asarray(res.results[i]["y"])[:, :g["nb"]] for i in range(NCORES)],
        axis=1)
    return np.ascontiguousarray(out.T.astype(np.float32))


# revision 28
# speedup vs baseline: 1.0280x; 1.0280x over previous
"""GENConv-style message passing + MLP head on 8 Trainium2 NeuronCores (Bass/Tile).

Sharding: destination nodes across 8 cores (each core owns its contiguous node
rows and its contiguous edge block). Weights replicated. The per-edge source
features arrive as a host-prepared shard layout: each core's input includes
x.T columns replicated per edge (xg[:, e] = x[src[e], :]), so the device
computes x_j = w_src @ xg[:, e] with a plain streaming matmul — no indirect
DMA (measured ~8 ns/row descriptor generation on the Q7 makes device-side row
gathers ~1.6 ms/core, far slower than streaming the expanded columns).

Edge stage (layout: channels on partitions, edges on the free dim, two
512-edge halves stacked on partition halves so DVE/ACT run 128 lanes wide;
one super-tile = 2048 edges = a [128, 1024] f32 PSUM tile):
  v = w_src@xg + blockdiag(w_edge)@ea2 accumulated in PSUM (six matmuls per
  2048 edges), m = relu(v) and a = exp(m) on ACT, p = m*a on DVE, then
  per-node sums over K=32 edges are free-dim segmented DVE reductions.
  Softmax uses the no-max identity out = sum(m e^m)/sum(e^m) + eps + dst
  (exact: exp cannot overflow here, and the eps/gmax factors cancel).
  Invalid and padded edges have xg and ea zeroed, so they contribute m=0,
  a=1, p=0; the spurious a=1 counts are subtracted per node via a
  host-computed count.

MLP head: h = out @ w1.T with train-mode BatchNorm using global batch stats
(partial sums AllReduce'd across the 8 cores), relu, then @ w2.T.
"""

import numpy as np
import ml_dtypes

import concourse.bacc as bacc
import concourse.bass as bass
import concourse.mybir as mybir
import concourse.tile as tile
from concourse.bass_utils import run_bass_kernel_spmd

BF16 = ml_dtypes.bfloat16
dt = mybir.dt
P = 128

N, K, IN_C, H, ED = 50000, 32, 128, 64, 32
NCORES = 8
MSG_EPS = 1e-7
BN_EPS = 1e-5
TP = 1024  # edges per pair-tile (two 512-edge halves stacked on partitions)


def _geom(n, ncores):
    nb = n // ncores
    ngrp = -(-nb // 64)        # node groups of 64 (= one 2048-edge super-tile)
    np_pad = ngrp * 64
    return dict(n=n, ncores=ncores, nb=nb, np_pad=np_pad, ntile=ngrp,
                ep=np_pad * K)


def _mlp_chunks(nb):
    out = []
    c = 0
    while c < nb:
        out.append((c, min(512, nb - c)))
        c += 512
    return out


def build_program(g):
    n, ncores, nb, np_pad = g["n"], g["ncores"], g["nb"], g["np_pad"]
    ntile, ep = g["ntile"], g["ep"]

    nc = bacc.Bacc(None, target_bir_lowering=False, num_devices=ncores)

    xg_h = nc.dram_tensor("xg", [IN_C, ep], dt.bfloat16, kind="ExternalInput")
    ea2_h = nc.dram_tensor("ea2", [2 * ED, ep // 2], dt.bfloat16,
                           kind="ExternalInput")
    xTd_h = nc.dram_tensor("xTd", [IN_C, np_pad], dt.bfloat16,
                           kind="ExternalInput")
    cnt_h = nc.dram_tensor("cnt", [P, np_pad // 2], dt.float32,
                           kind="ExternalInput")
    wsrc_h = nc.dram_tensor("wsrcT", [IN_C, H], dt.bfloat16,
                            kind="ExternalInput")
    wdst_h = nc.dram_tensor("wdstT", [IN_C, H], dt.bfloat16,
                            kind="ExternalInput")
    wbd_h = nc.dram_tensor("wbd2", [2 * ED, P], dt.bfloat16,
                           kind="ExternalInput")
    w1_h = nc.dram_tensor("w1T", [H, 2 * H], dt.bfloat16, kind="ExternalInput")
    w2_h = nc.dram_tensor("w2T", [2 * H, H], dt.bfloat16, kind="ExternalInput")
    gam_h = nc.dram_tensor("gamma", [2 * H, 1], dt.float32,
                           kind="ExternalInput")
    bet_h = nc.dram_tensor("beta", [2 * H, 1], dt.float32, kind="ExternalInput")
    y_h = nc.dram_tensor("y", [H, np_pad], dt.float32, kind="ExternalOutput")

    chunks = _mlp_chunks(nb)
    nch = len(chunks)

    with tile.TileContext(nc) as tc:
        with tc.tile_pool(name="const", bufs=1) as cpool, \
             tc.tile_pool(name="xgin", bufs=3) as xgpool, \
             tc.tile_pool(name="eain", bufs=3) as eapool, \
             tc.tile_pool(name="work", bufs=8) as wpool, \
             tc.tile_pool(name="node", bufs=4) as npool, \
             tc.tile_pool(name="psV", bufs=3, space="PSUM") as psV, \
             tc.tile_pool(name="psB", bufs=2, space="PSUM") as psB, \
             tc.tile_pool(name="dram", bufs=1, space="DRAM") as dpool:

            # ---- resident constants ----
            wsrc_t = cpool.tile([IN_C, H], dt.bfloat16)
            wdst_t = cpool.tile([IN_C, H], dt.bfloat16)
            wbd_t = cpool.tile([2 * ED, P], dt.bfloat16)
            w1_t = cpool.tile([H, 2 * H], dt.bfloat16)
            w2_t = cpool.tile([2 * H, H], dt.bfloat16)
            gam_t = cpool.tile([2 * H, 1], dt.float32)
            bet_t = cpool.tile([2 * H, 1], dt.float32)
            eps_t = cpool.tile([P, 1], dt.float32)
            bne_t = cpool.tile([P, 1], dt.float32)
            nc.sync.dma_start(wsrc_t[:], wsrc_h[:])
            nc.sync.dma_start(wdst_t[:], wdst_h[:])
            nc.sync.dma_start(wbd_t[:], wbd_h[:])
            nc.sync.dma_start(w1_t[:], w1_h[:])
            nc.sync.dma_start(w2_t[:], w2_h[:])
            nc.sync.dma_start(gam_t[:], gam_h[:])
            nc.sync.dma_start(bet_t[:], bet_h[:])
            nc.vector.memset(eps_t[:], MSG_EPS)
            nc.vector.memset(bne_t[:], BN_EPS)

            cnt_sb = cpool.tile([P, np_pad // 2], dt.float32)  # stacked counts
            dstk_sb = cpool.tile([P, np_pad // 2], dt.float32)  # denom stacked
            nstk_sb = cpool.tile([P, np_pad // 2], dt.float32)  # num stacked
            ostk_sb = cpool.tile([P, np_pad // 2], dt.bfloat16)  # out stacked
            outT_sb = cpool.tile([H, np_pad], dt.bfloat16)
            h_sb = cpool.tile([P, nb], dt.bfloat16)
            nc.sync.dma_start(cnt_sb[:], cnt_h[:])

            bnc_in = dpool.tile([2 * H, 2], dt.float32)
            bnc_out = dpool.tile([2 * H, 2], dt.float32)

            # ---- dst features (+ eps) written straight into outT ----
            c = 0
            while c < np_pad:
                w = min(512, np_pad - c)
                xd = xgpool.tile([IN_C, 512], dt.bfloat16, tag="xd")
                nc.sync.dma_start(xd[:, :w], xTd_h[:, c:c + w])
                ps = psB.tile([P, 512], dt.float32, tag="big")
                nc.tensor.matmul(ps[0:H, :w], lhsT=wdst_t[:], rhs=xd[:, :w],
                                 start=True, stop=True)
                nc.scalar.activation(outT_sb[:, c:c + w], ps[0:H, :w],
                                     mybir.ActivationFunctionType.Identity,
                                     bias=eps_t[0:H, 0:1])
                c += w

            # ---- edge stage: one super-tile = 2048 edges = 64 nodes ----
            T2 = 2 * TP
            for t in range(ntile):
                e0 = t * T2
                xgt = xgpool.tile([IN_C, T2], dt.bfloat16, tag="xg")
                nc.sync.dma_start(xgt[:], xg_h[:, e0:e0 + T2])
                eat = eapool.tile([2 * ED, TP], dt.bfloat16, tag="ea")
                nc.sync.dma_start(eat[:], ea2_h[:, e0 // 2:(e0 + T2) // 2])

                vps = psV.tile([P, 1024], dt.float32, tag="v")
                for hh in range(2):
                    o5, o10 = hh * 512, hh * 1024
                    nc.tensor.matmul(vps[:, o5:o5 + 512],
                                     lhsT=wbd_t[:], rhs=eat[:, o5:o5 + 512],
                                     start=True, stop=False,
                                     skip_group_check=True)
                    nc.tensor.matmul(vps[0:H, o5:o5 + 512], lhsT=wsrc_t[:],
                                     rhs=xgt[:, o10:o10 + 512], start=False,
                                     stop=False, skip_group_check=True)
                    nc.tensor.matmul(vps[H:P, o5:o5 + 512], lhsT=wsrc_t[:],
                                     rhs=xgt[:, o10 + 512:o10 + 1024],
                                     start=False, stop=True,
                                     skip_group_check=True)

                m_t = wpool.tile([P, 1024], dt.bfloat16, tag="m")
                nc.scalar.activation(m_t[:], vps[:],
                                     mybir.ActivationFunctionType.Relu)
                a_t = wpool.tile([P, 1024], dt.bfloat16, tag="a")
                nc.scalar.activation(a_t[:], m_t[:],
                                     mybir.ActivationFunctionType.Exp)
                p_t = wpool.tile([P, 1024], dt.bfloat16, tag="p")
                nc.vector.tensor_tensor(out=p_t[:], in0=m_t[:], in1=a_t[:],
                                        op=mybir.AluOpType.mult)
                c0 = t * 32
                nc.vector.reduce_sum(dstk_sb[:, c0:c0 + 32],
                                     a_t[:].rearrange("p (s k) -> p s k", k=K),
                                     axis=mybir.AxisListType.X)
                nc.vector.reduce_sum(nstk_sb[:, c0:c0 + 32],
                                     p_t[:].rearrange("p (s k) -> p s k", k=K),
                                     axis=mybir.AxisListType.X)

            # ---- node stage in stacked layout ----
            c = 0
            while c < np_pad // 2:
                w = min(512, np_pad // 2 - c)
                dn = npool.tile([P, 512], dt.float32, tag="dn")
                nc.vector.tensor_tensor(out=dn[:, :w], in0=dstk_sb[:, c:c + w],
                                        in1=cnt_sb[:, c:c + w],
                                        op=mybir.AluOpType.subtract)
                rr = npool.tile([P, 512], dt.float32, tag="rr")
                nc.vector.reciprocal(rr[:, :w], dn[:, :w])
                nc.vector.tensor_tensor(out=ostk_sb[:, c:c + w],
                                        in0=nstk_sb[:, c:c + w],
                                        in1=rr[:, :w], op=mybir.AluOpType.mult)
                c += w

            # ---- unstack node sums, then add into outT (dst already there) ----
            ou_sb = cpool.tile([H, np_pad], dt.bfloat16)
            for src_row, off in ((0, 0), (H, 16)):
                nc.sync.dma_start(
                    ou_sb[:].rearrange("h (t s) -> h t s", s=32)[:, :, off:off + 16],
                    ostk_sb[src_row:src_row + H, :].rearrange(
                        "h (t s) -> h t s", s=16))
            c = 0
            while c < np_pad:
                w = min(512, np_pad - c)
                nc.vector.tensor_tensor(out=outT_sb[:, c:c + w],
                                        in0=outT_sb[:, c:c + w],
                                        in1=ou_sb[:, c:c + w],
                                        op=mybir.AluOpType.add)
                c += w

            # ---- MLP + BatchNorm (global stats via AllReduce) ----
            st_s = cpool.tile([P, nch], dt.float32)
            st_q = cpool.tile([P, nch], dt.float32)
            for ci, (c, w) in enumerate(chunks):
                hps = psB.tile([P, 512], dt.float32, tag="big")
                nc.tensor.matmul(hps[:, :w], lhsT=w1_t[:],
                                 rhs=outT_sb[:, c:c + w], start=True, stop=True)
                nc.scalar.copy(h_sb[:, c:c + w], hps[:, :w])
                nc.vector.reduce_sum(st_s[:, ci:ci + 1], hps[:, :w],
                                     axis=mybir.AxisListType.X)
                sqd = wpool.tile([P, 512], dt.bfloat16, tag="sqd")
                nc.scalar.activation(sqd[:, :w], hps[:, :w],
                                     mybir.ActivationFunctionType.Square,
                                     accum_out=st_q[:, ci:ci + 1])
            pk = cpool.tile([P, 2], dt.float32)
            nc.vector.reduce_sum(pk[:, 0:1], st_s[:, 0:nch],
                                 axis=mybir.AxisListType.X)
            nc.vector.reduce_sum(pk[:, 1:2], st_q[:, 0:nch],
                                 axis=mybir.AxisListType.X)
            nc.gpsimd.dma_start(bnc_in[:], pk[:])
            nc.gpsimd.collective_compute(
                "AllReduce", mybir.AluOpType.add,
                replica_groups=[list(range(ncores))],
                ins=[bnc_in.opt()], outs=[bnc_out.opt()],
            )
            sg = cpool.tile([P, 2], dt.float32)
            nc.gpsimd.dma_start(sg[:], bnc_out[:])

            mean = cpool.tile([P, 1], dt.float32)
            ex2 = cpool.tile([P, 1], dt.float32)
            var = cpool.tile([P, 1], dt.float32)
            sd = cpool.tile([P, 1], dt.float32)
            inv = cpool.tile([P, 1], dt.float32)
            scl = cpool.tile([P, 1], dt.float32)
            tmp = cpool.tile([P, 1], dt.float32)
            shf = cpool.tile([P, 1], dt.float32)
            nc.vector.tensor_scalar_mul(mean[:], sg[:, 0:1], 1.0 / n)
            nc.vector.tensor_scalar_mul(ex2[:], sg[:, 1:2], 1.0 / n)
            nc.vector.tensor_tensor(out=tmp[:], in0=mean[:], in1=mean[:],
                                    op=mybir.AluOpType.mult)
            nc.vector.tensor_tensor(out=var[:], in0=ex2[:], in1=tmp[:],
                                    op=mybir.AluOpType.subtract)
            nc.scalar.activation(sd[:], var[:],
                                 mybir.ActivationFunctionType.Sqrt,
                                 bias=bne_t[:, 0:1])
            nc.vector.reciprocal(inv[:], sd[:])
            nc.vector.tensor_tensor(out=scl[:], in0=gam_t[:], in1=inv[:],
                                    op=mybir.AluOpType.mult)
            nc.vector.tensor_tensor(out=tmp[:], in0=mean[:], in1=scl[:],
                                    op=mybir.AluOpType.mult)
            nc.vector.tensor_tensor(out=shf[:], in0=bet_t[:], in1=tmp[:],
                                    op=mybir.AluOpType.subtract)

            for ci, (c, w) in enumerate(chunks):
                hr = wpool.tile([P, 512], dt.bfloat16, tag="hr")
                nc.scalar.activation(hr[:, :w], h_sb[:, c:c + w],
                                     mybir.ActivationFunctionType.Relu,
                                     scale=scl[:, 0:1], bias=shf[:, 0:1])
                yps = psB.tile([P, 512], dt.float32, tag="big")
                nc.tensor.matmul(yps[0:H, :w], lhsT=w2_t[:], rhs=hr[:, :w],
                                 start=True, stop=True)
                yo = npool.tile([H, 512], dt.float32, tag="yo")
                nc.vector.tensor_copy(yo[:, :w], yps[0:H, :w])
                nc.sync.dma_start(y_h[:, c:c + w], yo[:, :w])

    nc.compile()
    return nc


def prep_inputs(g, x, edge_attr, w_src, w_dst, w_edge, w1, gamma, beta, w2,
                edge_index, nbr):
    """Host-side shard prep. Returns list of per-core input dicts."""
    n, ncores, nb, np_pad = g["n"], g["ncores"], g["nb"], g["np_pad"]
    ep = g["ep"]
    k = nbr.shape[1]

    x = np.asarray(x, np.float32)
    edge_attr = np.asarray(edge_attr, np.float32)
    nbr = np.asarray(nbr)
    src_n = np.asarray(edge_index[0])

    # The shard layout relies on the consistent-graph structure from
    # setup_inputs: edge e has destination e // K and nbr[n] lists node n's
    # contiguous edge ids (-1-padded). Fail loudly if that ever changes.
    e_ids = np.arange(nbr.size, dtype=nbr.dtype).reshape(nbr.shape)
    valid = nbr >= 0
    assert np.array_equal(np.where(valid, nbr, -1), np.where(valid, e_ids, -1)), \
        "nbr is not the contiguous edge-id layout this kernel assumes"
    dst_n = np.asarray(edge_index[1])
    assert np.array_equal(dst_n[:: nbr.shape[1]],
                          np.arange(nbr.shape[0], dtype=dst_n.dtype)), \
        "edge destinations are not grouped contiguously by node"
    deg = valid.sum(1).astype(np.float32)
    cnt = (k - deg).astype(np.float32)

    xT = np.ascontiguousarray(x.T).astype(BF16)
    wsrcT = np.ascontiguousarray(w_src.T).astype(BF16)
    wdstT = np.ascontiguousarray(w_dst.T).astype(BF16)
    w1T = np.ascontiguousarray(np.asarray(w1).T).astype(BF16)
    w2T = np.ascontiguousarray(np.asarray(w2).T).astype(BF16)
    weT = np.asarray(w_edge, np.float32).T          # [ED, H]
    wbd2 = np.zeros((2 * ED, P), np.float32)
    wbd2[0:ED, 0:H] = weT
    wbd2[ED:2 * ED, H:P] = weT
    wbd2 = wbd2.astype(BF16)
    gam = np.asarray(gamma, np.float32).reshape(2 * H, 1)
    bet = np.asarray(beta, np.float32).reshape(2 * H, 1)

    maps = []
    for i in range(ncores):
        n0 = i * nb
        val_b = valid[n0:n0 + nb].reshape(-1)
        src_b = np.asarray(src_n[n0 * k:(n0 + nb) * k], np.int64)

        # xg: per-edge source columns, zeroed for invalid/padded edges
        src_pad = np.zeros(ep, np.int64)
        src_pad[:nb * k] = np.where(val_b, src_b, 0)
        xg = xT[:, src_pad]
        xg[:, :nb * k][:, ~val_b] = 0
        xg[:, nb * k:] = 0

        ea_b = np.zeros((ep, ED), np.float32)
        ea_b[:nb * k] = np.where(val_b[:, None],
                                 edge_attr[n0 * k:(n0 + nb) * k], 0.0)
        # stacked pairs: col (u*512+c) rows 0:32 = ea[1024u+c], 32:64 = +512
        ea2 = np.ascontiguousarray(
            ea_b.reshape(-1, 2, 512, ED).transpose(1, 3, 0, 2).reshape(
                2 * ED, ep // 2)).astype(BF16)

        cnt_core = np.zeros(np_pad, np.float32)
        cnt_core[:nb] = cnt[n0:n0 + nb]
        cc = cnt_core.reshape(-1, 2, 16)  # [t, half, s]
        cntT = np.ascontiguousarray(np.concatenate([
            np.broadcast_to(cc[:, 0, :].reshape(1, -1), (H, np_pad // 2)),
            np.broadcast_to(cc[:, 1, :].reshape(1, -1), (H, np_pad // 2)),
        ], axis=0)).astype(np.float32)

        xTd = np.zeros((IN_C, np_pad), BF16)
        xTd[:, :nb] = xT[:, n0:n0 + nb]

        maps.append({
            "xg": np.ascontiguousarray(xg), "ea2": ea2, "xTd": xTd,
            "cnt": cntT, "wsrcT": wsrcT, "wdstT": wdstT, "wbd2": wbd2,
            "w1T": w1T, "w2T": w2T, "gamma": gam, "beta": bet,
        })
    return maps


def kernel(x, edge_attr, w_src, w_dst, w_edge, w1, gamma, beta, w2, edge_index,
           nbr):
    import os
    g = _geom(N, NCORES)
    maps = prep_inputs(g, x, edge_attr, w_src, w_dst, w_edge, w1, gamma, beta,
                       w2, edge_index, nbr)
    nc = build_program(g)
    trace = bool(os.environ.get("GEN_KERNEL_TRACE"))
    res = run_bass_kernel_spmd(nc, maps, core_ids=list(range(NCORES)),
                               trace=trace)
    if trace:
        kernel.last_result = res
    out = np.concatenate(
        [np.asarray(res.results[i]["y"])[:, :g["nb"]] for i in range(NCORES)],
        axis=1)
    return np.ascontiguousarray(out.T.astype(np.float32))


# revision 29
# speedup vs baseline: 1.0947x; 1.0649x over previous
"""GENConv-style message passing + MLP head on 8 Trainium2 NeuronCores (Bass/Tile).

Sharding: destination nodes across 8 cores (each core owns its contiguous node
rows and its contiguous edge block). Weights replicated. The per-edge source
features arrive as a host-prepared shard layout: each core's input includes
x.T columns replicated per edge (xg[:, e] = x[src[e], :]), so the device
computes x_j = w_src @ xg[:, e] with a plain streaming matmul — no indirect
DMA (measured ~8 ns/row descriptor generation on the Q7 makes device-side row
gathers ~1.6 ms/core, far slower than streaming the expanded columns).

Edge stage (layout: channels on partitions, edges on the free dim, two
512-edge halves stacked on partition halves so DVE/ACT run 128 lanes wide;
one super-tile = 2048 edges = a [128, 1024] f32 PSUM tile):
  v = w_src@xg + blockdiag(w_edge)@ea2 accumulated in PSUM (six matmuls per
  2048 edges), m = relu(v) and a = exp(m) on ACT, p = m*a on DVE, then
  per-node sums over K=32 edges are free-dim segmented DVE reductions.
  Softmax uses the no-max identity out = sum(m e^m)/sum(e^m) + eps + dst
  (exact: exp cannot overflow here, and the eps/gmax factors cancel).
  Invalid and padded edges have xg and ea zeroed, so they contribute m=0,
  a=1, p=0; the spurious a=1 counts are subtracted per node via a
  host-computed count.

MLP head: h = out @ w1.T with train-mode BatchNorm using global batch stats
(partial sums AllReduce'd across the 8 cores), relu, then @ w2.T.
"""

import numpy as np
import ml_dtypes

import concourse.bacc as bacc
import concourse.bass as bass
import concourse.mybir as mybir
import concourse.tile as tile
from concourse.bass_utils import run_bass_kernel_spmd

BF16 = ml_dtypes.bfloat16
dt = mybir.dt
P = 128

N, K, IN_C, H, ED = 50000, 32, 128, 64, 32
NCORES = 8
MSG_EPS = 1e-7
BN_EPS = 1e-5
TP = 1024  # edges per pair-tile (two 512-edge halves stacked on partitions)


def _geom(n, ncores):
    nb = n // ncores
    ngrp = -(-nb // 64)        # node groups of 64 (= one 2048-edge super-tile)
    np_pad = ngrp * 64
    return dict(n=n, ncores=ncores, nb=nb, np_pad=np_pad, ntile=ngrp,
                ep=np_pad * K)


def _mlp_chunks(nb):
    out = []
    c = 0
    while c < nb:
        out.append((c, min(512, nb - c)))
        c += 512
    return out


def build_program(g):
    n, ncores, nb, np_pad = g["n"], g["ncores"], g["nb"], g["np_pad"]
    ntile, ep = g["ntile"], g["ep"]

    nc = bacc.Bacc(None, target_bir_lowering=False, num_devices=ncores)

    xg_h = nc.dram_tensor("xg", [IN_C, ep], dt.bfloat16, kind="ExternalInput")
    ea2_h = nc.dram_tensor("ea2", [2 * ED, ep // 2], dt.bfloat16,
                           kind="ExternalInput")
    xTd_h = nc.dram_tensor("xTd", [IN_C, np_pad], dt.bfloat16,
                           kind="ExternalInput")
    cnt_h = nc.dram_tensor("cnt", [P, np_pad // 2], dt.float32,
                           kind="ExternalInput")
    wsrc_h = nc.dram_tensor("wsrcT", [IN_C, H], dt.bfloat16,
                            kind="ExternalInput")
    wdst_h = nc.dram_tensor("wdstT", [IN_C, H], dt.bfloat16,
                            kind="ExternalInput")
    wbd_h = nc.dram_tensor("wbd2", [2 * ED, P], dt.bfloat16,
                           kind="ExternalInput")
    w1_h = nc.dram_tensor("w1T", [H, 2 * H], dt.bfloat16, kind="ExternalInput")
    w2_h = nc.dram_tensor("w2T", [2 * H, H], dt.bfloat16, kind="ExternalInput")
    gam_h = nc.dram_tensor("gamma", [2 * H, 1], dt.float32,
                           kind="ExternalInput")
    bet_h = nc.dram_tensor("beta", [2 * H, 1], dt.float32, kind="ExternalInput")
    y_h = nc.dram_tensor("y", [H, np_pad], dt.float32, kind="ExternalOutput")

    chunks = _mlp_chunks(nb)
    nch = len(chunks)

    with tile.TileContext(nc) as tc:
        with tc.tile_pool(name="const", bufs=1) as cpool, \
             tc.tile_pool(name="xgin", bufs=3) as xgpool, \
             tc.tile_pool(name="eain", bufs=3) as eapool, \
             tc.tile_pool(name="work", bufs=8) as wpool, \
             tc.tile_pool(name="node", bufs=4) as npool, \
             tc.tile_pool(name="psV", bufs=3, space="PSUM") as psV, \
             tc.tile_pool(name="psB", bufs=2, space="PSUM") as psB, \
             tc.tile_pool(name="dram", bufs=1, space="DRAM") as dpool:

            # ---- resident constants ----
            wsrc_t = cpool.tile([IN_C, H], dt.bfloat16)
            wdst_t = cpool.tile([IN_C, H], dt.bfloat16)
            wbd_t = cpool.tile([2 * ED, P], dt.bfloat16)
            w1_t = cpool.tile([H, 2 * H], dt.bfloat16)
            w2_t = cpool.tile([2 * H, H], dt.bfloat16)
            gam_t = cpool.tile([2 * H, 1], dt.float32)
            bet_t = cpool.tile([2 * H, 1], dt.float32)
            eps_t = cpool.tile([P, 1], dt.float32)
            bne_t = cpool.tile([P, 1], dt.float32)
            nc.sync.dma_start(wsrc_t[:], wsrc_h[:])
            nc.sync.dma_start(wdst_t[:], wdst_h[:])
            nc.sync.dma_start(wbd_t[:], wbd_h[:])
            nc.sync.dma_start(w1_t[:], w1_h[:])
            nc.sync.dma_start(w2_t[:], w2_h[:])
            nc.sync.dma_start(gam_t[:], gam_h[:])
            nc.sync.dma_start(bet_t[:], bet_h[:])
            nc.vector.memset(eps_t[:], MSG_EPS)
            nc.vector.memset(bne_t[:], BN_EPS)

            cnt_sb = cpool.tile([P, np_pad // 2], dt.float32)  # stacked counts
            dstk_sb = cpool.tile([P, np_pad // 2], dt.float32)  # denom stacked
            nstk_sb = cpool.tile([P, np_pad // 2], dt.float32)  # num stacked
            ostk_sb = cpool.tile([P, np_pad // 2], dt.bfloat16)  # out stacked
            outT_sb = cpool.tile([H, np_pad], dt.bfloat16)
            h_sb = cpool.tile([P, nb], dt.bfloat16)
            nc.sync.dma_start(cnt_sb[:], cnt_h[:])

            bnc_in = dpool.tile([2 * H, 2], dt.float32)
            bnc_out = dpool.tile([2 * H, 2], dt.float32)

            # ---- dst features (+ eps) written straight into outT ----
            c = 0
            while c < np_pad:
                w = min(512, np_pad - c)
                xd = xgpool.tile([IN_C, 512], dt.bfloat16, tag="xd")
                nc.sync.dma_start(xd[:, :w], xTd_h[:, c:c + w])
                ps = psB.tile([P, 512], dt.float32, tag="big")
                nc.tensor.matmul(ps[0:H, :w], lhsT=wdst_t[:], rhs=xd[:, :w],
                                 start=True, stop=True)
                nc.scalar.activation(outT_sb[:, c:c + w], ps[0:H, :w],
                                     mybir.ActivationFunctionType.Identity,
                                     bias=eps_t[0:H, 0:1])
                c += w

            # ---- edge stage: one super-tile = 2048 edges = 64 nodes ----
            T2 = 2 * TP
            for t in range(ntile):
                e0 = t * T2
                xgt = xgpool.tile([IN_C, T2], dt.bfloat16, tag="xg")
                nc.sync.dma_start(xgt[:], xg_h[:, e0:e0 + T2])
                eat = eapool.tile([2 * ED, TP], dt.bfloat16, tag="ea")
                nc.sync.dma_start(eat[:], ea2_h[:, e0 // 2:(e0 + T2) // 2])

                vps = psV.tile([P, 1024], dt.float32, tag="v")
                for hh in range(2):
                    o5, o10 = hh * 512, hh * 1024
                    nc.tensor.matmul(vps[:, o5:o5 + 512],
                                     lhsT=wbd_t[:], rhs=eat[:, o5:o5 + 512],
                                     start=True, stop=False,
                                     skip_group_check=True)
                    nc.tensor.matmul(vps[0:H, o5:o5 + 512], lhsT=wsrc_t[:],
                                     rhs=xgt[:, o10:o10 + 512], start=False,
                                     stop=False, skip_group_check=True)
                    nc.tensor.matmul(vps[H:P, o5:o5 + 512], lhsT=wsrc_t[:],
                                     rhs=xgt[:, o10 + 512:o10 + 1024],
                                     start=False, stop=True,
                                     skip_group_check=True)

                m_t = wpool.tile([P, 1024], dt.bfloat16, tag="m")
                nc.scalar.activation(m_t[:], vps[:],
                                     mybir.ActivationFunctionType.Relu)
                a_t = wpool.tile([P, 1024], dt.bfloat16, tag="a")
                nc.scalar.activation(a_t[:], m_t[:],
                                     mybir.ActivationFunctionType.Exp)
                p_t = wpool.tile([P, 1024], dt.bfloat16, tag="p")
                nc.gpsimd.tensor_tensor(out=p_t[:], in0=m_t[:], in1=a_t[:],
                                        op=mybir.AluOpType.mult)
                c0 = t * 32
                nc.vector.reduce_sum(dstk_sb[:, c0:c0 + 32],
                                     a_t[:].rearrange("p (s k) -> p s k", k=K),
                                     axis=mybir.AxisListType.X)
                nc.vector.reduce_sum(nstk_sb[:, c0:c0 + 32],
                                     p_t[:].rearrange("p (s k) -> p s k", k=K),
                                     axis=mybir.AxisListType.X)

            # ---- node stage in stacked layout ----
            c = 0
            while c < np_pad // 2:
                w = min(512, np_pad // 2 - c)
                dn = npool.tile([P, 512], dt.float32, tag="dn")
                nc.vector.tensor_tensor(out=dn[:, :w], in0=dstk_sb[:, c:c + w],
                                        in1=cnt_sb[:, c:c + w],
                                        op=mybir.AluOpType.subtract)
                rr = npool.tile([P, 512], dt.float32, tag="rr")
                nc.vector.reciprocal(rr[:, :w], dn[:, :w])
                nc.vector.tensor_tensor(out=ostk_sb[:, c:c + w],
                                        in0=nstk_sb[:, c:c + w],
                                        in1=rr[:, :w], op=mybir.AluOpType.mult)
                c += w

            # ---- unstack node sums, then add into outT (dst already there) ----
            ou_sb = cpool.tile([H, np_pad], dt.bfloat16)
            for src_row, off in ((0, 0), (H, 16)):
                nc.sync.dma_start(
                    ou_sb[:].rearrange("h (t s) -> h t s", s=32)[:, :, off:off + 16],
                    ostk_sb[src_row:src_row + H, :].rearrange(
                        "h (t s) -> h t s", s=16))
            c = 0
            while c < np_pad:
                w = min(512, np_pad - c)
                nc.vector.tensor_tensor(out=outT_sb[:, c:c + w],
                                        in0=outT_sb[:, c:c + w],
                                        in1=ou_sb[:, c:c + w],
                                        op=mybir.AluOpType.add)
                c += w

            # ---- MLP + BatchNorm (global stats via AllReduce) ----
            st_s = cpool.tile([P, nch], dt.float32)
            st_q = cpool.tile([P, nch], dt.float32)
            for ci, (c, w) in enumerate(chunks):
                hps = psB.tile([P, 512], dt.float32, tag="big")
                nc.tensor.matmul(hps[:, :w], lhsT=w1_t[:],
                                 rhs=outT_sb[:, c:c + w], start=True, stop=True)
                nc.scalar.copy(h_sb[:, c:c + w], hps[:, :w])
                nc.vector.reduce_sum(st_s[:, ci:ci + 1], hps[:, :w],
                                     axis=mybir.AxisListType.X)
                sqd = wpool.tile([P, 512], dt.bfloat16, tag="sqd")
                nc.scalar.activation(sqd[:, :w], hps[:, :w],
                                     mybir.ActivationFunctionType.Square,
                                     accum_out=st_q[:, ci:ci + 1])
            pk = cpool.tile([P, 2], dt.float32)
            nc.vector.reduce_sum(pk[:, 0:1], st_s[:, 0:nch],
                                 axis=mybir.AxisListType.X)
            nc.vector.reduce_sum(pk[:, 1:2], st_q[:, 0:nch],
                                 axis=mybir.AxisListType.X)
            nc.gpsimd.dma_start(bnc_in[:], pk[:])
            nc.gpsimd.collective_compute(
                "AllReduce", mybir.AluOpType.add,
                replica_groups=[list(range(ncores))],
                ins=[bnc_in.opt()], outs=[bnc_out.opt()],
            )
            sg = cpool.tile([P, 2], dt.float32)
            nc.gpsimd.dma_start(sg[:], bnc_out[:])

            mean = cpool.tile([P, 1], dt.float32)
            ex2 = cpool.tile([P, 1], dt.float32)
            var = cpool.tile([P, 1], dt.float32)
            sd = cpool.tile([P, 1], dt.float32)
            inv = cpool.tile([P, 1], dt.float32)
            scl = cpool.tile([P, 1], dt.float32)
            tmp = cpool.tile([P, 1], dt.float32)
            shf = cpool.tile([P, 1], dt.float32)
            nc.vector.tensor_scalar_mul(mean[:], sg[:, 0:1], 1.0 / n)
            nc.vector.tensor_scalar_mul(ex2[:], sg[:, 1:2], 1.0 / n)
            nc.vector.tensor_tensor(out=tmp[:], in0=mean[:], in1=mean[:],
                                    op=mybir.AluOpType.mult)
            nc.vector.tensor_tensor(out=var[:], in0=ex2[:], in1=tmp[:],
                                    op=mybir.AluOpType.subtract)
            nc.scalar.activation(sd[:], var[:],
                                 mybir.ActivationFunctionType.Sqrt,
                                 bias=bne_t[:, 0:1])
            nc.vector.reciprocal(inv[:], sd[:])
            nc.vector.tensor_tensor(out=scl[:], in0=gam_t[:], in1=inv[:],
                                    op=mybir.AluOpType.mult)
            nc.vector.tensor_tensor(out=tmp[:], in0=mean[:], in1=scl[:],
                                    op=mybir.AluOpType.mult)
            nc.vector.tensor_tensor(out=shf[:], in0=bet_t[:], in1=tmp[:],
                                    op=mybir.AluOpType.subtract)

            for ci, (c, w) in enumerate(chunks):
                hr = wpool.tile([P, 512], dt.bfloat16, tag="hr")
                nc.scalar.activation(hr[:, :w], h_sb[:, c:c + w],
                                     mybir.ActivationFunctionType.Relu,
                                     scale=scl[:, 0:1], bias=shf[:, 0:1])
                yps = psB.tile([P, 512], dt.float32, tag="big")
                nc.tensor.matmul(yps[0:H, :w], lhsT=w2_t[:], rhs=hr[:, :w],
                                 start=True, stop=True)
                yo = npool.tile([H, 512], dt.float32, tag="yo")
                nc.vector.tensor_copy(yo[:, :w], yps[0:H, :w])
                nc.sync.dma_start(y_h[:, c:c + w], yo[:, :w])

    nc.compile()
    return nc


def prep_inputs(g, x, edge_attr, w_src, w_dst, w_edge, w1, gamma, beta, w2,
                edge_index, nbr):
    """Host-side shard prep. Returns list of per-core input dicts."""
    n, ncores, nb, np_pad = g["n"], g["ncores"], g["nb"], g["np_pad"]
    ep = g["ep"]
    k = nbr.shape[1]

    x = np.asarray(x, np.float32)
    edge_attr = np.asarray(edge_attr, np.float32)
    nbr = np.asarray(nbr)
    src_n = np.asarray(edge_index[0])

    # The shard layout relies on the consistent-graph structure from
    # setup_inputs: edge e has destination e // K and nbr[n] lists node n's
    # contiguous edge ids (-1-padded). Fail loudly if that ever changes.
    e_ids = np.arange(nbr.size, dtype=nbr.dtype).reshape(nbr.shape)
    valid = nbr >= 0
    assert np.array_equal(np.where(valid, nbr, -1), np.where(valid, e_ids, -1)), \
        "nbr is not the contiguous edge-id layout this kernel assumes"
    dst_n = np.asarray(edge_index[1])
    assert np.array_equal(dst_n[:: nbr.shape[1]],
                          np.arange(nbr.shape[0], dtype=dst_n.dtype)), \
        "edge destinations are not grouped contiguously by node"
    deg = valid.sum(1).astype(np.float32)
    cnt = (k - deg).astype(np.float32)

    xT = np.ascontiguousarray(x.T).astype(BF16)
    wsrcT = np.ascontiguousarray(w_src.T).astype(BF16)
    wdstT = np.ascontiguousarray(w_dst.T).astype(BF16)
    w1T = np.ascontiguousarray(np.asarray(w1).T).astype(BF16)
    w2T = np.ascontiguousarray(np.asarray(w2).T).astype(BF16)
    weT = np.asarray(w_edge, np.float32).T          # [ED, H]
    wbd2 = np.zeros((2 * ED, P), np.float32)
    wbd2[0:ED, 0:H] = weT
    wbd2[ED:2 * ED, H:P] = weT
    wbd2 = wbd2.astype(BF16)
    gam = np.asarray(gamma, np.float32).reshape(2 * H, 1)
    bet = np.asarray(beta, np.float32).reshape(2 * H, 1)

    maps = []
    for i in range(ncores):
        n0 = i * nb
        val_b = valid[n0:n0 + nb].reshape(-1)
        src_b = np.asarray(src_n[n0 * k:(n0 + nb) * k], np.int64)

        # xg: per-edge source columns, zeroed for invalid/padded edges
        src_pad = np.zeros(ep, np.int64)
        src_pad[:nb * k] = np.where(val_b, src_b, 0)
        xg = xT[:, src_pad]
        xg[:, :nb * k][:, ~val_b] = 0
        xg[:, nb * k:] = 0

        ea_b = np.zeros((ep, ED), np.float32)
        ea_b[:nb * k] = np.where(val_b[:, None],
                                 edge_attr[n0 * k:(n0 + nb) * k], 0.0)
        # stacked pairs: col (u*512+c) rows 0:32 = ea[1024u+c], 32:64 = +512
        ea2 = np.ascontiguousarray(
            ea_b.reshape(-1, 2, 512, ED).transpose(1, 3, 0, 2).reshape(
                2 * ED, ep // 2)).astype(BF16)

        cnt_core = np.zeros(np_pad, np.float32)
        cnt_core[:nb] = cnt[n0:n0 + nb]
        cc = cnt_core.reshape(-1, 2, 16)  # [t, half, s]
        cntT = np.ascontiguousarray(np.concatenate([
            np.broadcast_to(cc[:, 0, :].reshape(1, -1), (H, np_pad // 2)),
            np.broadcast_to(cc[:, 1, :].reshape(1, -1), (H, np_pad // 2)),
        ], axis=0)).astype(np.float32)

        xTd = np.zeros((IN_C, np_pad), BF16)
        xTd[:, :nb] = xT[:, n0:n0 + nb]

        maps.append({
            "xg": np.ascontiguousarray(xg), "ea2": ea2, "xTd": xTd,
            "cnt": cntT, "wsrcT": wsrcT, "wdstT": wdstT, "wbd2": wbd2,
            "w1T": w1T, "w2T": w2T, "gamma": gam, "beta": bet,
        })
    return maps


def kernel(x, edge_attr, w_src, w_dst, w_edge, w1, gamma, beta, w2, edge_index,
           nbr):
    import os
    g = _geom(N, NCORES)
    maps = prep_inputs(g, x, edge_attr, w_src, w_dst, w_edge, w1, gamma, beta,
                       w2, edge_index, nbr)
    nc = build_program(g)
    trace = bool(os.environ.get("GEN_KERNEL_TRACE"))
    res = run_bass_kernel_spmd(nc, maps, core_ids=list(range(NCORES)),
                               trace=trace)
    if trace:
        kernel.last_result = res
    out = np.concatenate(
        [np.asarray(res.results[i]["y"])[:, :g["nb"]] for i in range(NCORES)],
        axis=1)
    return np.ascontiguousarray(out.T.astype(np.float32))
